# revision 25
# baseline (speedup 1.0000x reference)
"""Trainium2 Bass kernel for nn_DeChunkLayer (segment-reset linear scan + dechunk gather).

Math (from the reference):
    p  = clip(p_selected, EPS, 1-EPS);  dt = -log1p(-p)
    y_t = a_t * y_{t-1} + b_t  with  a_t = exp(-dt_t) (0 at segment starts),
                                     b_t = (dt_t*p_t) * (h_t/dt_t)  (~= p_t*h_t)
    out[j] = y[cumsum(b_flat)[j]-1]    (each outer row ~duplicated; host gather)

Device strategy (8 NeuronCores, sequence-parallel at segment boundaries):
  - Each core gets a contiguous token range starting at a segment boundary
    (fresh scan state), chopped into chunks of up to C=127 tokens.  A chunk
    is cut early when (a) its segment ends (no chunk crosses a segment
    boundary) or (b) its dt-sum would exceed RANGE_MAX (see below); the
    host-computed carry state flows chunk to chunk, so short chunks are
    just padding, not error.
  - Per chunk the scan is ONE bf16 matmul  y = M^T @ rhs.  The chunk
    coefficient matrix factorizes rank-1 over a constant causal mask:
        M[k,t] = u_k * L[k,t] * v_t,
        u_0 = 1 (carry row), u_{1+i} = p_i*exp(c_i - K),  v_t = exp(K - c_t),
    where c is the in-chunk inclusive dt-cumsum and K = clip(c_max-75, 0, 78).
    The RANGE_MAX=150 dt-sum cap keeps every factor inside f32/bf16 exponent
    range.  So instead of DMA-ing a [128,127] M per chunk (25% of load
    traffic in the previous version), the device loads TWO f32 scalars per
    token (u,v; one small DMA at startup) and builds  Lu = u .* L_const  on
    DVE/ACT ([128,127] tensor_scalar); the v scale rides the PSUM->SBUF copy
    for free (activation/tensor_scalar with per-partition scale).  rhs row 0
    is the HOST-computed exact chunk-boundary state pre-scaled by exp(-K).
  - DMA layout: every load/store is a row-slice of a DRAM tensor, i.e. a
    fully CONTIGUOUS region, and every HWDGE load tile has EXACTLY 128
    partitions: the HW DGE only splits a DIRECT2D transfer across the 16
    SDMA engines when the partition count divides evenly (128 = 16*8); a
    113-row tile pinned every load to ONE engine at 27 GB/s (measured:
    5.8x slowdown).
  - h and y travel as bf16 (matmul accumulates f32 in PSUM; norm rel-err
    ~3e-3 vs the f32 reference, tolerance is 2e-2).
"""

import numpy as np
import ml_dtypes

import concourse.bass as bass
import concourse.tile as tile
from concourse import mybir
from concourse.bass_utils import run_bass_kernel_spmd

EPS = 1e-4
N_CORES = 8
D = 512
C = 127          # max tokens per chunk (matrix row 0 is the carry row)
R = C + 1
BATCH = 6        # chunks per DMA batch (descriptor = BATCH*D*2 = 6 KB per
                 # row; 132 chunks = 22 batches exactly, so no padding)
RANGE_MAX = 150.0   # max in-chunk dt-sum for the rank-1 exp factors

F32 = mybir.dt.float32
BF16 = mybir.dt.bfloat16

_prog_cache: dict = {}
last_results = None  # BassKernelResults of the most recent device run (for test harness)


def _legalize_waits(nc: bass.Bass) -> None:
    """walrus codegen allows one sync-wait per engine instruction; move any
    surplus waits onto injected same-engine no-ops right before it."""
    nid = 0
    for fn in nc.m.functions:
        for blk in fn.blocks:
            out = []
            changed = False
            for inst in blk.instructions:
                si = getattr(inst, "sync_info", None)
                waits = list(si.on_wait) if si is not None and si.on_wait else []
                if len(waits) > 1:
                    for w in waits[:-1]:
                        nop = mybir.InstNoOp(
                            name=f"waitnop-{nid}", text_hint="waitsplit"
                        )
                        nid += 1
                        nop.engine = inst.engine
                        nop.sync_info = mybir.SyncInfo(on_wait=[w], on_update=[])
                        out.append(nop)
                    inst.sync_info = mybir.SyncInfo(
                        on_wait=[waits[-1]], on_update=list(si.on_update)
                    )
                    changed = True
                out.append(inst)
            if changed:
                blk.instructions = out


def _build_program(nchunk: int) -> bass.Bass:
    nbatch = nchunk // BATCH
    assert nchunk % BATCH == 0
    nc = bass.Bass("TRN2", target_bir_lowering=False, debug=False, num_devices=N_CORES)
    # row-major DRAM; batch b owns rows [b*R,(b+1)*R) -> every DMA below
    # moves one fully contiguous DRAM region with 128 partitions (see
    # module docstring: both properties are required for engine spreading)
    h_dev = nc.dram_tensor("h_dev", [nbatch * R, BATCH * D], BF16, kind="ExternalInput")
    uv_dev = nc.dram_tensor("uv_dev", [R, 2 * nchunk], F32, kind="ExternalInput")
    l_dev = nc.dram_tensor("l_dev", [R, C], BF16, kind="ExternalInput")
    # out rows per batch are padded 127 -> 128 (row 127 is a dummy) so the
    # store is a single 128-partition HWDGE transfer that spreads across all
    # 16 SDMA engines; host drops the dummy rows.  SWDGE (gpsimd) stores ran
    # ~11% slower per descriptor and added ~40 bookkeeping descriptors per
    # dispatch.
    out = nc.dram_tensor("out", [nbatch * R, BATCH * D], BF16, kind="ExternalOutput")

    with tile.TileContext(nc) as tc:
        with (
            tc.tile_pool(name="const", bufs=1) as cpool,
            tc.tile_pool(name="hpool", bufs=4) as hpool,
            tc.tile_pool(name="lupool", bufs=6) as lupool,
            tc.tile_pool(name="ypool", bufs=4) as ypool,
            tc.tile_pool(name="py", bufs=6, space="PSUM") as py,
        ):
            # one-time scalar/const loads on the Activation HWDGE ring so
            # they don't head-block the first h load on the sync ring
            uv_t = cpool.tile([R, 2 * nchunk], F32, tag="uv")
            nc.scalar.dma_start(uv_t, uv_dev[:, :])
            l_t = cpool.tile([R, C], BF16, tag="l")
            nc.scalar.dma_start(l_t, l_dev[:, :])

            for b in range(nbatch):
                rhs = hpool.tile([R, BATCH * D], BF16, tag="rhs")
                nc.sync.dma_start(rhs, h_dev[b * R : (b + 1) * R, :])
                y2 = ypool.tile([R, BATCH * D], BF16, tag="y2")
                # row 127 is a dummy (dropped by the host); it is stored
                # uninitialized on purpose -- a gpsimd memset to clear it
                # took 1.6us and sat in every batch's y2 dependency chain,
                # throttling the pipeline cadence to 3.3us/batch
                for ci in range(BATCH):
                    gc = b * BATCH + ci
                    u_ap = uv_t[:, 2 * gc : 2 * gc + 1]
                    v_ap = uv_t[0:C, 2 * gc + 1 : 2 * gc + 2]
                    lu = lupool.tile([R, C], BF16, tag="lu")
                    yp = py.tile([C, D], F32, tag="y")
                    dst = y2[0:C, ci * D : (ci + 1) * D]
                    # split the small Lu builds and big PSUM->SBUF copies
                    # between ACT and DVE; DVE gets 4 of 6 copies since its
                    # copy (~650ns) is faster than ACT's (~909ns), measured
                    # (gpsimd is far slower at both and cannot read PSUM)
                    act_copy = ci in (1, 4)
                    if act_copy:
                        nc.vector.tensor_scalar_mul(lu, l_t, u_ap)
                    else:
                        nc.scalar.mul(lu, l_t, u_ap)
                    nc.tensor.matmul(
                        yp,
                        lu,
                        rhs[:, ci * D : (ci + 1) * D],
                        start=True,
                        stop=True,
                    )
                    if act_copy:
                        nc.scalar.mul(dst, yp, v_ap)
                    else:
                        nc.vector.tensor_scalar_mul(dst, yp, v_ap)
                # one full-batch store per batch via SWDGE (gpsimd): SWDGE
                # pipelines dispatches, while a HWDGE ring processes roughly
                # one transfer at a time (execute + ~2us turnaround) and
                # falls behind the batch cadence, draining in a long
                # serialized tail (measured on two variants)
                nc.gpsimd.dma_start(out[b * R : (b + 1) * R, :], y2[:, :])
    _legalize_waits(nc)
    return nc


def _get_program(nchunk: int) -> bass.Bass:
    if nchunk not in _prog_cache:
        _prog_cache[nchunk] = _build_program(nchunk)
    return _prog_cache[nchunk]


def _split_ranges(starts: np.ndarray, length: int, k: int):
    """Partition [0,length) into k contiguous ranges cutting only at segment
    starts, minimizing the max range length. Returns list of (t0, t1)."""
    bounds = np.append(starts, length)
    lens = np.diff(bounds)
    nseg = len(lens)
    if nseg <= k:
        ranges = [(int(bounds[i]), int(bounds[i + 1])) for i in range(nseg)]
        ranges += [(length, length)] * (k - nseg)
        return ranges
    lo, hi = int(lens.max()), int(length)
    while lo < hi:
        mid = (lo + hi) // 2
        groups, cur = 1, 0
        for ln in lens:
            if cur + ln <= mid:
                cur += ln
            else:
                groups += 1
                cur = ln
        if groups <= k:
            hi = mid
        else:
            lo = mid + 1
    ranges = []
    s, cur = int(bounds[0]), 0
    for i, ln in enumerate(lens):
        if cur + ln > lo:
            ranges.append((s, int(bounds[i])))
            s, cur = int(bounds[i]), 0
        cur += int(ln)
    ranges.append((s, length))
    ranges += [(length, length)] * (k - len(ranges))
    return ranges


def _core_segments(starts: np.ndarray, t0: int, t1: int):
    """Segments [(s0,s1), ...] covering [t0,t1), cut at global segment starts."""
    if t1 <= t0:
        return []
    inner = starts[(starts > t0) & (starts < t1)]
    bounds = [t0] + [int(s) for s in inner] + [t1]
    return list(zip(bounds[:-1], bounds[1:]))


def _core_chunks(dt64, segs):
    """Greedy chunking: up to C tokens per chunk, cut early at segment ends
    and whenever the chunk dt-sum would exceed RANGE_MAX.  Returns
    ([(src0, src1)] per chunk, [segment-first flag per chunk])."""
    chunks = []
    first = []
    for s0, s1 in segs:
        i = s0
        at_start = True
        while i < s1:
            take = min(C, s1 - i)
            cs = np.cumsum(dt64[i : i + take])
            if cs[-1] > RANGE_MAX:
                take = int(np.searchsorted(cs, RANGE_MAX, side="right"))
            chunks.append((i, i + take))
            first.append(at_start)
            at_start = False
            i += take
    return chunks, first


def _core_inputs(h_flat, dt64, p64, chunks, seg_first, nchunk):
    """Build the per-core bf16 rhs tensor and f32 u/v scalar table.

    rhs chunk layout: row 0 = exp(-K)*S_prev (exact host f32 chunk-boundary
    state), rows 1..C = the chunk's tokens.  u/v as in the module docstring."""
    t_pad = nchunk * C

    dtl = np.zeros(t_pad)
    pl = np.zeros(t_pad)
    hl = np.zeros((t_pad, D), np.float32)
    firstf = np.zeros(nchunk, bool)
    for ci, (s0, s1) in enumerate(chunks):
        n = s1 - s0
        off = ci * C
        dtl[off : off + n] = dt64[s0:s1]
        pl[off : off + n] = p64[s0:s1]
        hl[off : off + n] = h_flat[s0:s1]
        firstf[ci] = seg_first[ci]

    dt2 = dtl.reshape(nchunk, C)
    p2 = pl.reshape(nchunk, C)
    h2 = hl.reshape(nchunk, C, D)

    c = dt2.cumsum(axis=1)                       # [n, C] in-chunk inclusive cumsum
    total = c[:, -1]
    K = np.clip(total - 75.0, 0.0, 78.0)
    u_tok = (p2 * np.exp(c - K[:, None])).astype(np.float32)
    v_tok = np.exp(K[:, None] - c).astype(np.float32)

    # exact chunk-boundary states: S_end = alpha*S_prev + z  (reset per segment)
    w = (p2 * np.exp(c - total[:, None])).astype(np.float32)
    z = np.einsum("nc,ncd->nd", w, h2)
    alpha = np.exp(-total)
    S_prev = np.zeros((nchunk, D), np.float32)
    s = np.zeros(D, np.float32)
    for ci in range(nchunk):
        if firstf[ci]:
            s = np.zeros(D, np.float32)
        S_prev[ci] = s
        s = (alpha[ci] * s + z[ci]).astype(np.float32)
    row0 = np.exp(-K)[:, None] * S_prev          # f64 scale, safe exponents

    bt = ml_dtypes.bfloat16
    nb = nchunk // BATCH
    hdev = np.zeros((nb, R, BATCH, D), np.float32)
    hdev[:, 0] = row0.reshape(nb, BATCH, D)
    hdev[:, 1:] = h2.reshape(nb, BATCH, C, D).transpose(0, 2, 1, 3)

    uv = np.zeros((R, 2 * nchunk), np.float32)
    uv[0, 0::2] = 1.0
    uv[1:, 0::2] = u_tok.T
    uv[0:C, 1::2] = v_tok.T
    return (
        np.ascontiguousarray(hdev.reshape(nb * R, BATCH * D)).astype(bt),
        np.ascontiguousarray(uv),
    )


def kernel(h_flat, b_flat, p_selected_flat, h_seq_idx):
    global last_results
    h_flat = np.ascontiguousarray(h_flat, np.float32)
    L, d = h_flat.shape
    assert d == D
    seg = np.asarray(h_seq_idx).reshape(-1).astype(np.int64)

    lo_f = np.float32(EPS)
    hi_f = np.float32(1.0 - EPS)
    p64 = np.clip(np.asarray(p_selected_flat, np.float32), lo_f, hi_f).astype(np.float64)
    dt64 = -np.log1p(-p64)

    startf = np.empty(L, bool)
    startf[0] = True
    startf[1:] = seg[1:] != seg[:-1]
    starts = np.flatnonzero(startf)

    idx = np.cumsum(np.asarray(b_flat, np.int64)) - 1

    ranges = _split_ranges(starts, L, N_CORES)
    core_chunks = []
    max_used = 1
    for t0, t1 in ranges:
        chunks, first = _core_chunks(dt64, _core_segments(starts, t0, t1))
        core_chunks.append((chunks, first))
        max_used = max(max_used, len(chunks))
    nchunk = -(-max_used // BATCH) * BATCH
    t_pad = nchunk * C
    nb = nchunk // BATCH

    nc = _get_program(nchunk)

    l_const = np.zeros((R, C), np.float32)
    l_const[0, :] = 1.0
    l_const[1:, :] = np.tri(C, C).T          # L[1+i,t] = 1 iff i <= t
    l_const = l_const.astype(ml_dtypes.bfloat16)

    in_maps = []
    for chunks, first in core_chunks:
        h_dev, uv_dev = _core_inputs(h_flat, dt64, p64, chunks, first, nchunk)
        in_maps.append({"h_dev": h_dev, "uv_dev": uv_dev, "l_dev": l_const})

    import os

    trace = bool(os.environ.get("BASSK_TRACE"))
    try:
        res = run_bass_kernel_spmd(
            nc, in_maps, core_ids=list(range(N_CORES)), trace=trace
        )
    except ModuleNotFoundError:
        res = run_bass_kernel_spmd(
            nc, in_maps, core_ids=list(range(N_CORES)), trace=False
        )
    last_results = res

    y = np.empty((L, D), np.float32)
    for i, (chunks, first) in enumerate(core_chunks):
        if not chunks:
            continue
        dev = np.asarray(res.results[i]["out"]).astype(np.float32)
        # [nb*R, BATCH*D]: row b*R+t (t<C), col ci*D: token (b*BATCH+ci)*C + t;
        # row b*R+127 is the store-padding dummy
        flat = (
            dev.reshape(nb, R, BATCH, D)[:, 0:C].transpose(0, 2, 1, 3).reshape(t_pad, D)
        )
        for ci, (s0, s1) in enumerate(chunks):
            y[s0:s1] = flat[ci * C : ci * C + (s1 - s0)]
    gidx = np.where(idx < 0, idx + L, idx)
    gidx = np.clip(gidx, 0, L - 1)
    return y[gidx]


# revision 26
# speedup vs baseline: 1.2106x; 1.2106x over previous
"""Trainium2 Bass kernel for nn_DeChunkLayer (segment-reset linear scan + dechunk gather).

Math (from the reference):
    p  = clip(p_selected, EPS, 1-EPS);  dt = -log1p(-p)
    y_t = a_t * y_{t-1} + b_t  with  a_t = exp(-dt_t) (0 at segment starts),
                                     b_t = (dt_t*p_t) * (h_t/dt_t)  (~= p_t*h_t)
    out[j] = y[cumsum(b_flat)[j]-1]    (each outer row ~duplicated; host gather)

Device strategy (8 NeuronCores, sequence-parallel at segment boundaries):
  - Each core gets a contiguous token range starting at a segment boundary
    (fresh scan state), chopped into chunks of up to C=127 tokens.  A chunk
    is cut early when (a) its segment ends (no chunk crosses a segment
    boundary) or (b) its dt-sum would exceed RANGE_MAX (see below); the
    host-computed carry state flows chunk to chunk, so short chunks are
    just padding, not error.
  - Per chunk the scan is ONE bf16 matmul  y = M^T @ rhs.  The chunk
    coefficient matrix factorizes rank-1 over a constant causal mask:
        M[k,t] = u_k * L[k,t] * v_t,
        u_0 = 1 (carry row), u_{1+i} = p_i*exp(c_i - K),  v_t = exp(K - c_t),
    where c is the in-chunk inclusive dt-cumsum and K = clip(c_max-75, 0, 78).
    The RANGE_MAX=150 dt-sum cap keeps every factor inside f32/bf16 exponent
    range.  So instead of DMA-ing a [128,127] M per chunk (25% of load
    traffic in the previous version), the device loads TWO f32 scalars per
    token (u,v; one small DMA at startup) and builds  Lu = u .* L_const  on
    DVE/ACT ([128,127] tensor_scalar); the v scale rides the PSUM->SBUF copy
    for free (activation/tensor_scalar with per-partition scale).  rhs row 0
    is the HOST-computed exact chunk-boundary state pre-scaled by exp(-K).
  - DMA layout: every load/store is a row-slice of a DRAM tensor, i.e. a
    fully CONTIGUOUS region, and every HWDGE load tile has EXACTLY 128
    partitions: the HW DGE only splits a DIRECT2D transfer across the 16
    SDMA engines when the partition count divides evenly (128 = 16*8); a
    113-row tile pinned every load to ONE engine at 27 GB/s (measured:
    5.8x slowdown).
  - h and y travel as bf16 (matmul accumulates f32 in PSUM; norm rel-err
    ~3e-3 vs the f32 reference, tolerance is 2e-2).
"""

import numpy as np
import ml_dtypes

import concourse.bass as bass
import concourse.tile as tile
from concourse import mybir
from concourse.bass_utils import run_bass_kernel_spmd

EPS = 1e-4
N_CORES = 8
D = 512
C = 127          # max tokens per chunk (matrix row 0 is the carry row)
R = C + 1
BATCH = 6        # chunks per DMA batch (descriptor = BATCH*D*2 = 6 KB per
                 # row; 132 chunks = 22 batches exactly, so no padding)
RANGE_MAX = 150.0   # max in-chunk dt-sum for the rank-1 exp factors

F32 = mybir.dt.float32
BF16 = mybir.dt.bfloat16

_prog_cache: dict = {}
last_results = None  # BassKernelResults of the most recent device run (for test harness)


def _legalize_waits(nc: bass.Bass) -> None:
    """walrus codegen allows one sync-wait per engine instruction; move any
    surplus waits onto injected same-engine no-ops right before it."""
    nid = 0
    for fn in nc.m.functions:
        for blk in fn.blocks:
            out = []
            changed = False
            for inst in blk.instructions:
                si = getattr(inst, "sync_info", None)
                waits = list(si.on_wait) if si is not None and si.on_wait else []
                if len(waits) > 1:
                    for w in waits[:-1]:
                        nop = mybir.InstNoOp(
                            name=f"waitnop-{nid}", text_hint="waitsplit"
                        )
                        nid += 1
                        nop.engine = inst.engine
                        nop.sync_info = mybir.SyncInfo(on_wait=[w], on_update=[])
                        out.append(nop)
                    inst.sync_info = mybir.SyncInfo(
                        on_wait=[waits[-1]], on_update=list(si.on_update)
                    )
                    changed = True
                out.append(inst)
            if changed:
                blk.instructions = out


def _build_program(nchunk: int) -> bass.Bass:
    nbatch = nchunk // BATCH
    assert nchunk % BATCH == 0
    nc = bass.Bass("TRN2", target_bir_lowering=False, debug=False, num_devices=N_CORES)
    # row-major DRAM; batch b owns rows [b*R,(b+1)*R) -> every DMA below
    # moves one fully contiguous DRAM region with 128 partitions (see
    # module docstring: both properties are required for engine spreading)
    h_dev = nc.dram_tensor("h_dev", [nbatch * R, BATCH * D], BF16, kind="ExternalInput")
    uv_dev = nc.dram_tensor("uv_dev", [R, 2 * nchunk], F32, kind="ExternalInput")
    l_dev = nc.dram_tensor("l_dev", [R, C], BF16, kind="ExternalInput")
    # out rows per batch are padded 127 -> 128 (row 127 is a dummy) so the
    # store is a single 128-partition HWDGE transfer that spreads across all
    # 16 SDMA engines; host drops the dummy rows.  SWDGE (gpsimd) stores ran
    # ~11% slower per descriptor and added ~40 bookkeeping descriptors per
    # dispatch.
    out = nc.dram_tensor("out", [nbatch * R, BATCH * D], BF16, kind="ExternalOutput")

    with tile.TileContext(nc) as tc:
        with (
            tc.tile_pool(name="const", bufs=1) as cpool,
            tc.tile_pool(name="hpool", bufs=4) as hpool,
            tc.tile_pool(name="lupool", bufs=6) as lupool,
            tc.tile_pool(name="ypool", bufs=4) as ypool,
            tc.tile_pool(name="py", bufs=6, space="PSUM") as py,
        ):
            # one-time scalar/const loads first on the sync ring (tiny)
            uv_t = cpool.tile([R, 2 * nchunk], F32, tag="uv")
            nc.sync.dma_start(uv_t, uv_dev[:, :])
            l_t = cpool.tile([R, C], BF16, tag="l")
            nc.sync.dma_start(l_t, l_dev[:, :])

            for b in range(nbatch):
                rhs = hpool.tile([R, BATCH * D], BF16, tag="rhs")
                nc.sync.dma_start(rhs, h_dev[b * R : (b + 1) * R, :])
                y2 = ypool.tile([R, BATCH * D], BF16, tag="y2")
                # row 127 is a dummy (dropped by the host); it is stored
                # uninitialized on purpose -- a gpsimd memset to clear it
                # took 1.6us and sat in every batch's y2 dependency chain,
                # throttling the pipeline cadence to 3.3us/batch
                for ci in range(BATCH):
                    gc = b * BATCH + ci
                    u_ap = uv_t[:, 2 * gc : 2 * gc + 1]
                    v_ap = uv_t[0:C, 2 * gc + 1 : 2 * gc + 2]
                    lu = lupool.tile([R, C], BF16, tag="lu")
                    yp = py.tile([C, D], F32, tag="y")
                    dst = y2[0:C, ci * D : (ci + 1) * D]
                    # split the small Lu builds and big PSUM->SBUF copies
                    # between ACT and DVE; DVE gets 4 of 6 copies since its
                    # copy (~650ns) is faster than ACT's (~909ns), measured
                    # (gpsimd is far slower at both and cannot read PSUM)
                    act_copy = ci in (1, 4)
                    if act_copy:
                        nc.vector.tensor_scalar_mul(lu, l_t, u_ap)
                    else:
                        nc.scalar.mul(lu, l_t, u_ap)
                    nc.tensor.matmul(
                        yp,
                        lu,
                        rhs[:, ci * D : (ci + 1) * D],
                        start=True,
                        stop=True,
                    )
                    if act_copy:
                        nc.scalar.mul(dst, yp, v_ap)
                    else:
                        nc.vector.tensor_scalar_mul(dst, yp, v_ap)
                # one full-batch store per batch via SWDGE (gpsimd): SWDGE
                # pipelines dispatches, while a HWDGE ring processes roughly
                # one transfer at a time (execute + ~2us turnaround) and
                # falls behind the batch cadence, draining in a long
                # serialized tail (measured on two variants)
                nc.gpsimd.dma_start(out[b * R : (b + 1) * R, :], y2[:, :])
    _legalize_waits(nc)
    return nc


def _get_program(nchunk: int) -> bass.Bass:
    if nchunk not in _prog_cache:
        _prog_cache[nchunk] = _build_program(nchunk)
    return _prog_cache[nchunk]


def _split_ranges(starts: np.ndarray, length: int, k: int):
    """Partition [0,length) into k contiguous ranges cutting only at segment
    starts, minimizing the max range length. Returns list of (t0, t1)."""
    bounds = np.append(starts, length)
    lens = np.diff(bounds)
    nseg = len(lens)
    if nseg <= k:
        ranges = [(int(bounds[i]), int(bounds[i + 1])) for i in range(nseg)]
        ranges += [(length, length)] * (k - nseg)
        return ranges
    lo, hi = int(lens.max()), int(length)
    while lo < hi:
        mid = (lo + hi) // 2
        groups, cur = 1, 0
        for ln in lens:
            if cur + ln <= mid:
                cur += ln
            else:
                groups += 1
                cur = ln
        if groups <= k:
            hi = mid
        else:
            lo = mid + 1
    ranges = []
    s, cur = int(bounds[0]), 0
    for i, ln in enumerate(lens):
        if cur + ln > lo:
            ranges.append((s, int(bounds[i])))
            s, cur = int(bounds[i]), 0
        cur += int(ln)
    ranges.append((s, length))
    ranges += [(length, length)] * (k - len(ranges))
    return ranges


def _core_segments(starts: np.ndarray, t0: int, t1: int):
    """Segments [(s0,s1), ...] covering [t0,t1), cut at global segment starts."""
    if t1 <= t0:
        return []
    inner = starts[(starts > t0) & (starts < t1)]
    bounds = [t0] + [int(s) for s in inner] + [t1]
    return list(zip(bounds[:-1], bounds[1:]))


def _core_chunks(dt64, segs):
    """Greedy chunking: up to C tokens per chunk, cut early at segment ends
    and whenever the chunk dt-sum would exceed RANGE_MAX.  Returns
    ([(src0, src1)] per chunk, [segment-first flag per chunk])."""
    chunks = []
    first = []
    for s0, s1 in segs:
        i = s0
        at_start = True
        while i < s1:
            take = min(C, s1 - i)
            cs = np.cumsum(dt64[i : i + take])
            if cs[-1] > RANGE_MAX:
                take = int(np.searchsorted(cs, RANGE_MAX, side="right"))
            chunks.append((i, i + take))
            first.append(at_start)
            at_start = False
            i += take
    return chunks, first


def _core_inputs(h_flat, dt64, p64, chunks, seg_first, nchunk):
    """Build the per-core bf16 rhs tensor and f32 u/v scalar table.

    rhs chunk layout: row 0 = exp(-K)*S_prev (exact host f32 chunk-boundary
    state), rows 1..C = the chunk's tokens.  u/v as in the module docstring."""
    t_pad = nchunk * C

    dtl = np.zeros(t_pad)
    pl = np.zeros(t_pad)
    hl = np.zeros((t_pad, D), np.float32)
    firstf = np.zeros(nchunk, bool)
    for ci, (s0, s1) in enumerate(chunks):
        n = s1 - s0
        off = ci * C
        dtl[off : off + n] = dt64[s0:s1]
        pl[off : off + n] = p64[s0:s1]
        hl[off : off + n] = h_flat[s0:s1]
        firstf[ci] = seg_first[ci]

    dt2 = dtl.reshape(nchunk, C)
    p2 = pl.reshape(nchunk, C)
    h2 = hl.reshape(nchunk, C, D)

    c = dt2.cumsum(axis=1)                       # [n, C] in-chunk inclusive cumsum
    total = c[:, -1]
    K = np.clip(total - 75.0, 0.0, 78.0)
    u_tok = (p2 * np.exp(c - K[:, None])).astype(np.float32)
    v_tok = np.exp(K[:, None] - c).astype(np.float32)

    # exact chunk-boundary states: S_end = alpha*S_prev + z  (reset per segment)
    w = (p2 * np.exp(c - total[:, None])).astype(np.float32)
    z = np.einsum("nc,ncd->nd", w, h2)
    alpha = np.exp(-total)
    S_prev = np.zeros((nchunk, D), np.float32)
    s = np.zeros(D, np.float32)
    for ci in range(nchunk):
        if firstf[ci]:
            s = np.zeros(D, np.float32)
        S_prev[ci] = s
        s = (alpha[ci] * s + z[ci]).astype(np.float32)
    row0 = np.exp(-K)[:, None] * S_prev          # f64 scale, safe exponents

    bt = ml_dtypes.bfloat16
    nb = nchunk // BATCH
    hdev = np.zeros((nb, R, BATCH, D), np.float32)
    hdev[:, 0] = row0.reshape(nb, BATCH, D)
    hdev[:, 1:] = h2.reshape(nb, BATCH, C, D).transpose(0, 2, 1, 3)

    uv = np.zeros((R, 2 * nchunk), np.float32)
    uv[0, 0::2] = 1.0
    uv[1:, 0::2] = u_tok.T
    uv[0:C, 1::2] = v_tok.T
    return (
        np.ascontiguousarray(hdev.reshape(nb * R, BATCH * D)).astype(bt),
        np.ascontiguousarray(uv),
    )


def kernel(h_flat, b_flat, p_selected_flat, h_seq_idx):
    global last_results
    h_flat = np.ascontiguousarray(h_flat, np.float32)
    L, d = h_flat.shape
    assert d == D
    seg = np.asarray(h_seq_idx).reshape(-1).astype(np.int64)

    lo_f = np.float32(EPS)
    hi_f = np.float32(1.0 - EPS)
    p64 = np.clip(np.asarray(p_selected_flat, np.float32), lo_f, hi_f).astype(np.float64)
    dt64 = -np.log1p(-p64)

    startf = np.empty(L, bool)
    startf[0] = True
    startf[1:] = seg[1:] != seg[:-1]
    starts = np.flatnonzero(startf)

    idx = np.cumsum(np.asarray(b_flat, np.int64)) - 1

    ranges = _split_ranges(starts, L, N_CORES)
    core_chunks = []
    max_used = 1
    for t0, t1 in ranges:
        chunks, first = _core_chunks(dt64, _core_segments(starts, t0, t1))
        core_chunks.append((chunks, first))
        max_used = max(max_used, len(chunks))
    nchunk = -(-max_used // BATCH) * BATCH
    t_pad = nchunk * C
    nb = nchunk // BATCH

    nc = _get_program(nchunk)

    l_const = np.zeros((R, C), np.float32)
    l_const[0, :] = 1.0
    l_const[1:, :] = np.tri(C, C).T          # L[1+i,t] = 1 iff i <= t
    l_const = l_const.astype(ml_dtypes.bfloat16)

    in_maps = []
    for chunks, first in core_chunks:
        h_dev, uv_dev = _core_inputs(h_flat, dt64, p64, chunks, first, nchunk)
        in_maps.append({"h_dev": h_dev, "uv_dev": uv_dev, "l_dev": l_const})

    import os

    trace = bool(os.environ.get("BASSK_TRACE"))
    try:
        res = run_bass_kernel_spmd(
            nc, in_maps, core_ids=list(range(N_CORES)), trace=trace
        )
    except ModuleNotFoundError:
        res = run_bass_kernel_spmd(
            nc, in_maps, core_ids=list(range(N_CORES)), trace=False
        )
    last_results = res

    y = np.empty((L, D), np.float32)
    for i, (chunks, first) in enumerate(core_chunks):
        if not chunks:
            continue
        dev = np.asarray(res.results[i]["out"]).astype(np.float32)
        # [nb*R, BATCH*D]: row b*R+t (t<C), col ci*D: token (b*BATCH+ci)*C + t;
        # row b*R+127 is the store-padding dummy
        flat = (
            dev.reshape(nb, R, BATCH, D)[:, 0:C].transpose(0, 2, 1, 3).reshape(t_pad, D)
        )
        for ci, (s0, s1) in enumerate(chunks):
            y[s0:s1] = flat[ci * C : ci * C + (s1 - s0)]
    gidx = np.where(idx < 0, idx + L, idx)
    gidx = np.clip(gidx, 0, L - 1)
    return y[gidx]


# revision 27
# speedup vs baseline: 1.3338x; 1.1017x over previous
"""Trainium2 Bass kernel for nn_DeChunkLayer (segment-reset linear scan + dechunk gather).

Math (from the reference):
    p  = clip(p_selected, EPS, 1-EPS);  dt = -log1p(-p)
    y_t = a_t * y_{t-1} + b_t  with  a_t = exp(-dt_t) (0 at segment starts),
                                     b_t = (dt_t*p_t) * (h_t/dt_t)  (~= p_t*h_t)
    out[j] = y[cumsum(b_flat)[j]-1]    (each outer row ~duplicated; host gather)

Device strategy (8 NeuronCores, sequence-parallel at segment boundaries):
  - Each core gets a contiguous token range starting at a segment boundary
    (fresh scan state), chopped into chunks of up to C=127 tokens.  A chunk
    is cut early when (a) its segment ends (no chunk crosses a segment
    boundary) or (b) its dt-sum would exceed RANGE_MAX (see below); the
    host-computed carry state flows chunk to chunk, so short chunks are
    just padding, not error.
  - Per chunk the scan is ONE bf16 matmul  y = M^T @ rhs.  The chunk
    coefficient matrix factorizes rank-1 over a constant causal mask:
        M[k,t] = u_k * L[k,t] * v_t,
        u_0 = 1 (carry row), u_{1+i} = p_i*exp(c_i - K),  v_t = exp(K - c_t),
    where c is the in-chunk inclusive dt-cumsum and K = clip(c_max-75, 0, 78).
    The RANGE_MAX=150 dt-sum cap keeps every factor inside f32/bf16 exponent
    range.  So instead of DMA-ing a [128,127] M per chunk (25% of load
    traffic in the previous version), the device loads TWO f32 scalars per
    token (u,v; one small DMA at startup) and builds  Lu = u .* L_const  on
    DVE/ACT ([128,127] tensor_scalar); the v scale rides the PSUM->SBUF copy
    for free (activation/tensor_scalar with per-partition scale).  rhs row 0
    is the HOST-computed exact chunk-boundary state pre-scaled by exp(-K).
  - DMA layout: every load/store is a row-slice of a DRAM tensor, i.e. a
    fully CONTIGUOUS region, and every HWDGE load tile has EXACTLY 128
    partitions: the HW DGE only splits a DIRECT2D transfer across the 16
    SDMA engines when the partition count divides evenly (128 = 16*8); a
    113-row tile pinned every load to ONE engine at 27 GB/s (measured:
    5.8x slowdown).
  - h and y travel as bf16 (matmul accumulates f32 in PSUM; norm rel-err
    ~3e-3 vs the f32 reference, tolerance is 2e-2).
"""

import numpy as np
import ml_dtypes

import concourse.bass as bass
import concourse.tile as tile
from concourse import mybir
from concourse.bass_utils import run_bass_kernel_spmd

EPS = 1e-4
N_CORES = 8
D = 512
C = 127          # max tokens per chunk (matrix row 0 is the carry row)
R = C + 1
BATCH = 6        # chunks per DMA batch (descriptor = BATCH*D*2 = 6 KB per
                 # row; 132 chunks = 22 batches exactly, so no padding)
RANGE_MAX = 150.0   # max in-chunk dt-sum for the rank-1 exp factors

F32 = mybir.dt.float32
BF16 = mybir.dt.bfloat16

_prog_cache: dict = {}
last_results = None  # BassKernelResults of the most recent device run (for test harness)


def _legalize_waits(nc: bass.Bass) -> None:
    """walrus codegen allows one sync-wait per engine instruction; move any
    surplus waits onto injected same-engine no-ops right before it."""
    nid = 0
    for fn in nc.m.functions:
        for blk in fn.blocks:
            out = []
            changed = False
            for inst in blk.instructions:
                si = getattr(inst, "sync_info", None)
                waits = list(si.on_wait) if si is not None and si.on_wait else []
                if len(waits) > 1:
                    for w in waits[:-1]:
                        nop = mybir.InstNoOp(
                            name=f"waitnop-{nid}", text_hint="waitsplit"
                        )
                        nid += 1
                        nop.engine = inst.engine
                        nop.sync_info = mybir.SyncInfo(on_wait=[w], on_update=[])
                        out.append(nop)
                    inst.sync_info = mybir.SyncInfo(
                        on_wait=[waits[-1]], on_update=list(si.on_update)
                    )
                    changed = True
                out.append(inst)
            if changed:
                blk.instructions = out


def _build_program(nchunk: int) -> bass.Bass:
    nbatch = nchunk // BATCH
    assert nchunk % BATCH == 0
    nc = bass.Bass("TRN2", target_bir_lowering=False, debug=False, num_devices=N_CORES)
    # row-major DRAM; batch b owns rows [b*R,(b+1)*R) -> every DMA below
    # moves one fully contiguous DRAM region with 128 partitions (see
    # module docstring: both properties are required for engine spreading)
    h_dev = nc.dram_tensor("h_dev", [nbatch * R, BATCH * D], BF16, kind="ExternalInput")
    uv_dev = nc.dram_tensor("uv_dev", [R, 2 * nchunk], F32, kind="ExternalInput")
    l_dev = nc.dram_tensor("l_dev", [R, C], BF16, kind="ExternalInput")
    # out rows per batch are padded 127 -> 128 (row 127 is a dummy) so the
    # store is a single 128-partition HWDGE transfer that spreads across all
    # 16 SDMA engines; host drops the dummy rows.  SWDGE (gpsimd) stores ran
    # ~11% slower per descriptor and added ~40 bookkeeping descriptors per
    # dispatch.
    out = nc.dram_tensor("out", [nbatch * R, BATCH * D], BF16, kind="ExternalOutput")

    with tile.TileContext(nc) as tc:
        with (
            tc.tile_pool(name="const", bufs=1) as cpool,
            tc.tile_pool(name="hpool", bufs=4) as hpool,
            tc.tile_pool(name="lupool", bufs=6) as lupool,
            tc.tile_pool(name="ypool", bufs=4) as ypool,
            tc.tile_pool(name="py", bufs=6, space="PSUM") as py,
        ):
            # one-time scalar/const loads first on the sync ring (tiny)
            uv_t = cpool.tile([R, 2 * nchunk], F32, tag="uv")
            nc.sync.dma_start(uv_t, uv_dev[:, :])
            l_t = cpool.tile([R, C], BF16, tag="l")
            nc.sync.dma_start(l_t, l_dev[:, :])

            for b in range(nbatch):
                rhs = hpool.tile([R, BATCH * D], BF16, tag="rhs")
                nc.sync.dma_start(rhs, h_dev[b * R : (b + 1) * R, :])
                y2 = ypool.tile([R, BATCH * D], BF16, tag="y2")
                # row 127 is a dummy (dropped by the host); it is stored
                # uninitialized on purpose -- a gpsimd memset to clear it
                # took 1.6us and sat in every batch's y2 dependency chain,
                # throttling the pipeline cadence to 3.3us/batch
                for ci in range(BATCH):
                    gc = b * BATCH + ci
                    u_ap = uv_t[:, 2 * gc : 2 * gc + 1]
                    v_ap = uv_t[0:C, 2 * gc + 1 : 2 * gc + 2]
                    lu = lupool.tile([R, C], BF16, tag="lu")
                    yp = py.tile([C, D], F32, tag="y")
                    dst = y2[0:C, ci * D : (ci + 1) * D]
                    # alternate ACT/DVE between the small Lu build and the
                    # big PSUM->SBUF copy so both engines stay ~half loaded
                    # (gpsimd is far slower at both and cannot read PSUM)
                    if gc % 2 == 0:
                        nc.vector.tensor_scalar_mul(lu, l_t, u_ap)
                    else:
                        nc.scalar.mul(lu, l_t, u_ap)
                    nc.tensor.matmul(
                        yp,
                        lu,
                        rhs[:, ci * D : (ci + 1) * D],
                        start=True,
                        stop=True,
                    )
                    if gc % 2 == 0:
                        nc.scalar.mul(dst, yp, v_ap)
                    else:
                        nc.vector.tensor_scalar_mul(dst, yp, v_ap)
                # one full-batch store per batch via SWDGE (gpsimd): SWDGE
                # pipelines dispatches, while a HWDGE ring processes roughly
                # one transfer at a time (execute + ~2us turnaround) and
                # falls behind the batch cadence, draining in a long
                # serialized tail (measured on two variants)
                nc.gpsimd.dma_start(out[b * R : (b + 1) * R, :], y2[:, :])
    _legalize_waits(nc)
    return nc


def _get_program(nchunk: int) -> bass.Bass:
    if nchunk not in _prog_cache:
        _prog_cache[nchunk] = _build_program(nchunk)
    return _prog_cache[nchunk]


def _split_ranges(starts: np.ndarray, length: int, k: int):
    """Partition [0,length) into k contiguous ranges cutting only at segment
    starts, minimizing the max range length. Returns list of (t0, t1)."""
    bounds = np.append(starts, length)
    lens = np.diff(bounds)
    nseg = len(lens)
    if nseg <= k:
        ranges = [(int(bounds[i]), int(bounds[i + 1])) for i in range(nseg)]
        ranges += [(length, length)] * (k - nseg)
        return ranges
    lo, hi = int(lens.max()), int(length)
    while lo < hi:
        mid = (lo + hi) // 2
        groups, cur = 1, 0
        for ln in lens:
            if cur + ln <= mid:
                cur += ln
            else:
                groups += 1
                cur = ln
        if groups <= k:
            hi = mid
        else:
            lo = mid + 1
    ranges = []
    s, cur = int(bounds[0]), 0
    for i, ln in enumerate(lens):
        if cur + ln > lo:
            ranges.append((s, int(bounds[i])))
            s, cur = int(bounds[i]), 0
        cur += int(ln)
    ranges.append((s, length))
    ranges += [(length, length)] * (k - len(ranges))
    return ranges


def _core_segments(starts: np.ndarray, t0: int, t1: int):
    """Segments [(s0,s1), ...] covering [t0,t1), cut at global segment starts."""
    if t1 <= t0:
        return []
    inner = starts[(starts > t0) & (starts < t1)]
    bounds = [t0] + [int(s) for s in inner] + [t1]
    return list(zip(bounds[:-1], bounds[1:]))


def _core_chunks(dt64, segs):
    """Greedy chunking: up to C tokens per chunk, cut early at segment ends
    and whenever the chunk dt-sum would exceed RANGE_MAX.  Returns
    ([(src0, src1)] per chunk, [segment-first flag per chunk])."""
    chunks = []
    first = []
    for s0, s1 in segs:
        i = s0
        at_start = True
        while i < s1:
            take = min(C, s1 - i)
            cs = np.cumsum(dt64[i : i + take])
            if cs[-1] > RANGE_MAX:
                take = int(np.searchsorted(cs, RANGE_MAX, side="right"))
            chunks.append((i, i + take))
            first.append(at_start)
            at_start = False
            i += take
    return chunks, first


def _core_inputs(h_flat, dt64, p64, chunks, seg_first, nchunk):
    """Build the per-core bf16 rhs tensor and f32 u/v scalar table.

    rhs chunk layout: row 0 = exp(-K)*S_prev (exact host f32 chunk-boundary
    state), rows 1..C = the chunk's tokens.  u/v as in the module docstring."""
    t_pad = nchunk * C

    dtl = np.zeros(t_pad)
    pl = np.zeros(t_pad)
    hl = np.zeros((t_pad, D), np.float32)
    firstf = np.zeros(nchunk, bool)
    for ci, (s0, s1) in enumerate(chunks):
        n = s1 - s0
        off = ci * C
        dtl[off : off + n] = dt64[s0:s1]
        pl[off : off + n] = p64[s0:s1]
        hl[off : off + n] = h_flat[s0:s1]
        firstf[ci] = seg_first[ci]

    dt2 = dtl.reshape(nchunk, C)
    p2 = pl.reshape(nchunk, C)
    h2 = hl.reshape(nchunk, C, D)

    c = dt2.cumsum(axis=1)                       # [n, C] in-chunk inclusive cumsum
    total = c[:, -1]
    K = np.clip(total - 75.0, 0.0, 78.0)
    u_tok = (p2 * np.exp(c - K[:, None])).astype(np.float32)
    v_tok = np.exp(K[:, None] - c).astype(np.float32)

    # exact chunk-boundary states: S_end = alpha*S_prev + z  (reset per segment)
    w = (p2 * np.exp(c - total[:, None])).astype(np.float32)
    z = np.einsum("nc,ncd->nd", w, h2)
    alpha = np.exp(-total)
    S_prev = np.zeros((nchunk, D), np.float32)
    s = np.zeros(D, np.float32)
    for ci in range(nchunk):
        if firstf[ci]:
            s = np.zeros(D, np.float32)
        S_prev[ci] = s
        s = (alpha[ci] * s + z[ci]).astype(np.float32)
    row0 = np.exp(-K)[:, None] * S_prev          # f64 scale, safe exponents

    bt = ml_dtypes.bfloat16
    nb = nchunk // BATCH
    hdev = np.zeros((nb, R, BATCH, D), np.float32)
    hdev[:, 0] = row0.reshape(nb, BATCH, D)
    hdev[:, 1:] = h2.reshape(nb, BATCH, C, D).transpose(0, 2, 1, 3)

    uv = np.zeros((R, 2 * nchunk), np.float32)
    uv[0, 0::2] = 1.0
    uv[1:, 0::2] = u_tok.T
    uv[0:C, 1::2] = v_tok.T
    return (
        np.ascontiguousarray(hdev.reshape(nb * R, BATCH * D)).astype(bt),
        np.ascontiguousarray(uv),
    )


def kernel(h_flat, b_flat, p_selected_flat, h_seq_idx):
    global last_results
    h_flat = np.ascontiguousarray(h_flat, np.float32)
    L, d = h_flat.shape
    assert d == D
    seg = np.asarray(h_seq_idx).reshape(-1).astype(np.int64)

    lo_f = np.float32(EPS)
    hi_f = np.float32(1.0 - EPS)
    p64 = np.clip(np.asarray(p_selected_flat, np.float32), lo_f, hi_f).astype(np.float64)
    dt64 = -np.log1p(-p64)

    startf = np.empty(L, bool)
    startf[0] = True
    startf[1:] = seg[1:] != seg[:-1]
    starts = np.flatnonzero(startf)

    idx = np.cumsum(np.asarray(b_flat, np.int64)) - 1

    ranges = _split_ranges(starts, L, N_CORES)
    core_chunks = []
    max_used = 1
    for t0, t1 in ranges:
        chunks, first = _core_chunks(dt64, _core_segments(starts, t0, t1))
        core_chunks.append((chunks, first))
        max_used = max(max_used, len(chunks))
    nchunk = -(-max_used // BATCH) * BATCH
    t_pad = nchunk * C
    nb = nchunk // BATCH

    nc = _get_program(nchunk)

    l_const = np.zeros((R, C), np.float32)
    l_const[0, :] = 1.0
    l_const[1:, :] = np.tri(C, C).T          # L[1+i,t] = 1 iff i <= t
    l_const = l_const.astype(ml_dtypes.bfloat16)

    in_maps = []
    for chunks, first in core_chunks:
        h_dev, uv_dev = _core_inputs(h_flat, dt64, p64, chunks, first, nchunk)
        in_maps.append({"h_dev": h_dev, "uv_dev": uv_dev, "l_dev": l_const})

    import os

    trace = bool(os.environ.get("BASSK_TRACE"))
    try:
        res = run_bass_kernel_spmd(
            nc, in_maps, core_ids=list(range(N_CORES)), trace=trace
        )
    except ModuleNotFoundError:
        res = run_bass_kernel_spmd(
            nc, in_maps, core_ids=list(range(N_CORES)), trace=False
        )
    last_results = res

    y = np.empty((L, D), np.float32)
    for i, (chunks, first) in enumerate(core_chunks):
        if not chunks:
            continue
        dev = np.asarray(res.results[i]["out"]).astype(np.float32)
        # [nb*R, BATCH*D]: row b*R+t (t<C), col ci*D: token (b*BATCH+ci)*C + t;
        # row b*R+127 is the store-padding dummy
        flat = (
            dev.reshape(nb, R, BATCH, D)[:, 0:C].transpose(0, 2, 1, 3).reshape(t_pad, D)
        )
        for ci, (s0, s1) in enumerate(chunks):
            y[s0:s1] = flat[ci * C : ci * C + (s1 - s0)]
    gidx = np.where(idx < 0, idx + L, idx)
    gidx = np.clip(gidx, 0, L - 1)
    return y[gidx]
